# revision 41
# baseline (speedup 1.0000x reference)
"""
Trainium2 Bass kernel for a lower-triangular-masked GRU.

Math (per reference):
  lower = tril(ones(H,H)); WiG' = W_iG*lower + diag(b_iG); WhG' = W_hG*lower
  r = sigmoid(x @ Wir' + h @ Whr' + b_hr)
  z = sigmoid(x @ Wiz' + h @ Whz' + b_hz)
  n = tanh(x @ Win' + r * (h @ Whn' + b_hn))
  h' = h*z + (1-z)*n
  label = sigmoid(h' * W_out + b_out) * x ; ans[t,b] = max_h label >= 0.5 ? 1 : -1

Strategy: data-parallel over batch (B=64 -> 8 per core). Everything on
device runs in "hT layout": H on partitions (8 k-blocks of 128), batch on
the free dim, so the scan state needs no transposes. All matmuls are fp16
(1 cycle/row vs 4 for fp32; the values are bounded, so fp16's 0.05%
rounding is safe — verified exact-match on hardware).

The x-projections are fused into the scan: for every 8-step sub-window the
bulk matmuls x @ Wi' (+ the b_h{r,z} bias seeds, via identity matmul of a
pre-replicated bias tile) accumulate into a per-gate PSUM tile
[128, KB, QW, BS]; the per-step recurrent matmuls then accumulate into the
same regions. This removes the separate bulk phase, the fp32 x input and
the 50MB pre round-trip through DRAM that serialized on the DMA queue.

The recurrent matmuls use split-h: by linearity W.T@h' = W.T@q1 + W.T@q2
with q1 = z*h (ready mid-chain) and q2 = (1-z)*n (the chain tail), so the
h' = q1+q2 add is off the critical path and the next step's PE work gated
on q2 is only the r-gate block column. Per-step critical chain:
  r = sig(pr) [Act, in-place in PSUM] -> t1 = r*pn [DVE] -> t2 = t1+xn
  [DVE] -> n = tanh(t2) [Act] -> q2 [DVE] -> (PE r-gate q2-matmuls)
z, 1-z (= sigmoid(-pz)), q1 and the label path run off-chain; the label
tail (sigmoid on Act, *x on Pool, KB max tree on DVE) is batched per
8-step quarter; the max across partitions is finished on the host.
"""

import sys
import numpy as np
from contextlib import ExitStack

for _p in ("/opt/trn_rl_repo", "/root/.axon_site/_ro/trn_rl_repo"):
    if _p not in sys.path:
        sys.path.insert(0, _p)

import concourse.bass as bass
import concourse.tile as tile
from concourse import bacc
from concourse import mybir
from concourse.bass_utils import run_bass_kernel_spmd

T, B, H = 512, 64, 1024
NC = 8           # cores
BS = B // NC     # batch per core = 8
KB = H // 128    # 8 k-blocks
WIN = 32         # scan steps per For_i iteration
NW = T // WIN    # 16 windows
QW = 8           # sub-window size (steps): x-projection batch + label tail

F32 = mybir.dt.float32
F16 = mybir.dt.float16
AF = mybir.ActivationFunctionType
ALU = mybir.AluOpType

# sim.py sets this to unroll the scan loop so TimelineSim (no_exec=True)
# can resolve control flow without an interpreter. Hardware runs use the
# For_i hardware loop (smaller instruction memory footprint).
SIM_UNROLL = False

# perf measurement: repeat the whole compute R times on device (outer
# hardware loop). Output is overwritten identically each repeat.
REPEAT = 1

LAST_RESULT = None

# packed triangular weight layout: only the nonzero (j <= k) 128x128 blocks
# are stored, as [128, NPACK*128] slabs ordered (k, g, j<=k)
def _woff(k, g, j):
    return (3 * (k * (k + 1) // 2) + g * (k + 1) + j) * 128

NPACK = 3 * (KB * (KB + 1) // 2)  # 108 slabs


def _build(b_out: float):
    nc = bacc.Bacc(None)

    xt_d = nc.declare_dram_parameter("xt16", [KB, 128, T, BS], F16, isOutput=False)
    wih_d = nc.declare_dram_parameter("wih", [128, NPACK * 128], F16, isOutput=False)
    whh_d = nc.declare_dram_parameter("whh", [128, NPACK * 128], F16, isOutput=False)
    bsr_d = nc.declare_dram_parameter("biasr", [128, KB, QW, BS], F16, isOutput=False)
    bsz_d = nc.declare_dram_parameter("biasz", [128, KB, QW, BS], F16, isOutput=False)
    h0t_d = nc.declare_dram_parameter("h0t", [128, KB * BS], F16, isOutput=False)
    bhn_d = nc.declare_dram_parameter("bhn", [128, KB * BS], F16, isOutput=False)
    wout_d = nc.declare_dram_parameter("woutt", [128, KB * BS], F32, isOutput=False)
    eye_d = nc.declare_dram_parameter("eye", [128, 128], F16, isOutput=False)
    mbuf_d = nc.declare_dram_parameter("mbuf", [128, T, BS], F16, isOutput=True)

    with tile.TileContext(nc) as tc, ExitStack() as ctx:
        consts = ctx.enter_context(tc.tile_pool(name="consts", bufs=1))
        wpool = ctx.enter_context(tc.tile_pool(name="wpool", bufs=2))
        xpsum = ctx.enter_context(
            tc.tile_pool(name="xpsum", bufs=2, space=bass.MemorySpace.PSUM)
        )
        spsum = ctx.enter_context(
            tc.tile_pool(name="spsum", bufs=2, space=bass.MemorySpace.PSUM)
        )
        xwp = ctx.enter_context(tc.tile_pool(name="xwp", bufs=2))
        xnp = ctx.enter_context(tc.tile_pool(name="xnp", bufs=2))
        lbp = ctx.enter_context(tc.tile_pool(name="lbp", bufs=2))
        vbp = ctx.enter_context(tc.tile_pool(name="vbp", bufs=1))
        m1p = ctx.enter_context(tc.tile_pool(name="m1p", bufs=2))
        hp = ctx.enter_context(tc.tile_pool(name="hp", bufs=3))
        tp = ctx.enter_context(tc.tile_pool(name="tp", bufs=2))

        # both weight sets stay resident, triangular-packed (27KB each)
        wi = wpool.tile([128, NPACK * 128], F16)
        nc.sync.dma_start(out=wi[:], in_=wih_d[:])
        wh = wpool.tile([128, NPACK * 128], F16)
        nc.sync.dma_start(out=wh[:], in_=whh_d[:])

        eye = consts.tile([128, 128], F16)
        nc.sync.dma_start(out=eye[:], in_=eye_d[:])
        bconst = consts.tile([128, 1], F32)
        nc.vector.memset(bconst[:, 0:1], b_out)
        bhn = consts.tile([128, KB * BS], F16)
        nc.sync.dma_start(out=bhn[:], in_=bhn_d[:])
        bsr = consts.tile([128, KB, QW, BS], F16)
        nc.sync.dma_start(out=bsr[:], in_=bsr_d[:])
        bsz = consts.tile([128, KB, QW, BS], F16)
        nc.sync.dma_start(out=bsz[:], in_=bsz_d[:])
        woutt = consts.tile([128, KB * BS], F32)
        nc.sync.dma_start(out=woutt[:], in_=wout_d[:])

        # persistent scan state: h, and the two halves q1 = z*h, q2 = (1-z)*n
        # whose sum is h' (fed to the next step's matmuls separately)
        ht = hp.tile([128, KB * BS], F16)
        q1t = hp.tile([128, KB * BS], F16)
        q2t = hp.tile([128, KB * BS], F16)

        def emit_subwindow_x(xw, q):
            """Bulk x-projections + bias seeds for sub-window q into fresh
            per-gate PSUM tiles [128, KB, QW, BS]. Returns (xpr, xpz, xnq)
            where xnq is the n-gate x-part evacuated to SBUF fp16."""
            xpr = xpsum.tile([128, KB, QW, BS], F32)
            xpz = xpsum.tile([128, KB, QW, BS], F32)
            xpn = xpsum.tile([128, KB, QW, BS], F32)
            nc.tensor.matmul(xpr[:], eye[:], bsr[:], start=True, stop=False)
            nc.tensor.matmul(xpz[:], eye[:], bsz[:], start=True, stop=False)
            for g, xp in ((0, xpr), (1, xpz), (2, xpn)):
                for j in range(KB):
                    for k in range(j, KB):
                        nc.tensor.matmul(
                            xp[:, j, :, :],
                            wi[:, _woff(k, g, j) : _woff(k, g, j) + 128],
                            xw[:, k, q * QW : (q + 1) * QW, :],
                            start=(g == 2 and k == j),
                            stop=(g == 2 and k == KB - 1),
                        )
            # evacuate the n-gate x-part to SBUF fp16: the per-step
            # t2 = t1 + xn add then runs in the DVE 2x mode instead of
            # paying the PSUM access latency
            xnq = xnp.tile([128, KB, QW, BS], F16)
            nc.vector.tensor_copy(xnq[:], xpn[:])
            return xpr, xpz, xnq

        def emit_window(w):
            xw = xwp.tile([128, KB, WIN, BS], F16)
            nc.sync.dma_start(
                out=xw[:], in_=xt_d[:, :, bass.ts(w, WIN), :].transpose([1, 0, 2, 3])
            )
            vbw = vbp.tile([128, WIN, KB * BS], F16)
            m1 = m1p.tile([128, WIN, BS], F16)

            subx = {}

            for s in range(WIN):
                q, sl = divmod(s, QW)
                if sl == 0 and q == 0:
                    subx[0] = emit_subwindow_x(xw, 0)
                if sl == 1 and q < WIN // QW - 1:
                    # emit the next sub-window's bulk-x mid-sub-window so the
                    # PE absorbs it in per-step idle gaps
                    subx[q + 1] = emit_subwindow_x(xw, q + 1)
                xpr, xpz, xnq = subx[q]
                prs = xpr[:, :, sl, :]
                pzs = xpz[:, :, sl, :]

                pn = spsum.tile([128, KB * BS], F32)
                nc.tensor.matmul(pn[:], eye[:], bhn[:], start=True, stop=False)
                # recurrent matmuls, split-h: W.T@h = W.T@q1 + W.T@q2 (by
                # linearity). q1 = z*h is ready mid-chain of the previous
                # step, so only the q2 half (r-gate first) gates the next
                # step's start; the h' = q1+q2 add is off the critical path.
                for qt, last in ((q1t, False), (q2t, True)):
                    for g, ps in ((0, prs), (2, pn), (1, pzs)):
                        for j in range(KB):
                            tgt = ps[:, j * BS : (j + 1) * BS] if g == 2 else ps[:, j, :]
                            for k in range(j, KB):
                                nc.tensor.matmul(
                                    tgt,
                                    wh[:, _woff(k, g, j) : _woff(k, g, j) + 128],
                                    qt[:, k * BS : (k + 1) * BS],
                                    start=False,
                                    stop=(last and k == KB - 1),
                                )
                # critical chain: r -> t1 -> t2 -> n -> q2
                r = tp.tile([128, KB * BS], F16)
                nc.scalar.activation(
                    r[:].rearrange("p (j b) -> p j b", j=KB), prs, AF.Sigmoid
                )
                t1 = tp.tile([128, KB * BS], F16)
                nc.vector.tensor_mul(t1[:], r[:], pn[:])
                t2 = tp.tile([128, KB * BS], F16)
                nc.vector.tensor_add(
                    t2[:].rearrange("p (j b) -> p j b", j=KB),
                    t1[:].rearrange("p (j b) -> p j b", j=KB),
                    xnq[:, :, sl, :],
                )
                n_ = tp.tile([128, KB * BS], F16)
                nc.scalar.activation(n_[:], t2[:], AF.Tanh)
                # off-chain: z and 1-z (= sigmoid(-pz)) from psum
                z = tp.tile([128, KB * BS], F16)
                nc.scalar.activation(
                    z[:].rearrange("p (j b) -> p j b", j=KB), pzs, AF.Sigmoid
                )
                oz = tp.tile([128, KB * BS], F16)
                nc.scalar.activation(
                    oz[:].rearrange("p (j b) -> p j b", j=KB), pzs, AF.Sigmoid,
                    scale=-1.0,
                )
                nc.vector.tensor_mul(q1t[:], z[:], ht[:])
                nc.vector.tensor_mul(q2t[:], oz[:], n_[:])
                nc.vector.tensor_add(ht[:], q1t[:], q2t[:])
                # label path: store v = h'*W_out per step (Pool); the
                # sigmoid / *x / max are batched per 8-step quarter below
                nc.gpsimd.tensor_mul(vbw[:, s, :], ht[:], woutt[:])

                if sl == QW - 1:
                    q0 = s - (QW - 1)
                    # sigmoid in place over the quarter's v values
                    nc.scalar.activation(
                        vbw[:, q0 : s + 1, :], vbw[:, q0 : s + 1, :], AF.Sigmoid,
                        bias=bconst[:, 0:1],
                    )
                    lbq = lbp.tile([128, QW, KB, BS], F16)
                    nc.gpsimd.tensor_mul(
                        lbq[:],
                        vbw[:, q0 : s + 1, :].rearrange("p w (j b) -> p w j b", j=KB),
                        xw[:, :, q0 : s + 1, :].rearrange("p j w b -> p w j b"),
                    )
                    # max tree over the KB blocks
                    ma = m1p.tile([128, QW, 4, BS], F16)
                    nc.vector.tensor_tensor(
                        ma[:], lbq[:, :, 0:4, :], lbq[:, :, 4:8, :], op=ALU.max
                    )
                    mb = m1p.tile([128, QW, 2, BS], F16)
                    nc.vector.tensor_tensor(
                        mb[:], ma[:, :, 0:2, :], ma[:, :, 2:4, :], op=ALU.max
                    )
                    nc.vector.tensor_tensor(
                        m1[:, q0 : s + 1, :], mb[:, :, 0, :], mb[:, :, 1, :],
                        op=ALU.max,
                    )
            # output DMA from the Pool queue so it does not head-of-line
            # block the next window's input DMA on the SP queue
            nc.gpsimd.dma_start(out=mbuf_d[:, bass.ts(w, WIN), :], in_=m1[:])

        def emit_all():
            nc.sync.dma_start(out=ht[:], in_=h0t_d[:])
            nc.sync.dma_start(out=q1t[:], in_=h0t_d[:])
            nc.vector.memset(q2t[:], 0.0)
            if SIM_UNROLL:
                for w in range(NW):
                    emit_window(w)
            else:
                with tc.For_i(0, NW, 1) as w:
                    emit_window(w)

        if REPEAT > 1:
            with tc.For_i(0, REPEAT, 1):
                emit_all()
        else:
            emit_all()

    nc.compile()
    return nc


def kernel(
    input_, hidden0, W_ir, W_hr, W_iz, W_hz, W_in, W_hn,
    b_ir, b_hr, b_iz, b_hz, b_in, b_hn, W_out, b_out,
):
    input_ = np.ascontiguousarray(input_, dtype=np.float32)
    hidden0 = np.asarray(hidden0, dtype=np.float32)

    L = np.tril(np.ones((H, H), dtype=np.float32))

    def pack_tri(ws):  # list of 3 full [H, H] -> [128, NPACK*128] fp16
        out = np.zeros((128, NPACK * 128), dtype=np.float16)
        for k in range(KB):
            for g in range(3):
                for j in range(k + 1):
                    off = _woff(k, g, j)
                    out[:, off : off + 128] = ws[g][
                        k * 128 : (k + 1) * 128, j * 128 : (j + 1) * 128
                    ]
        return out

    wih = pack_tri([
        W_ir * L + np.diag(b_ir),
        W_iz * L + np.diag(b_iz),
        W_in * L + np.diag(b_in),
    ])
    whh = pack_tri([W_hr * L, W_hz * L, W_hn * L])

    def rep_ht(vec, dt):  # [H] -> [128, KB*BS] hT-layout replicated over batch
        return np.repeat(
            vec.reshape(KB, 128).T[:, :, None], BS, axis=2
        ).reshape(128, KB * BS).astype(dt)

    bhn_t = rep_ht(b_hn, np.float16)
    wout_t = rep_ht(W_out, np.float32)
    # bias b_h{r,z} replicated over (QW, BS) for the PSUM bias seed
    def rep_bias(vec):
        return np.broadcast_to(
            vec.reshape(KB, 128).T[:, :, None, None], (128, KB, QW, BS)
        ).astype(np.float16)

    biasr = np.ascontiguousarray(rep_bias(b_hr))
    biasz = np.ascontiguousarray(rep_bias(b_hz))
    eye = np.eye(128, dtype=np.float16)

    nc = _build(float(np.asarray(b_out).reshape(-1)[0]))

    in_maps = []
    for c in range(NC):
        xc = input_[:, c * BS : (c + 1) * BS, :]  # [T, BS, H]
        xt16 = (
            np.ascontiguousarray(xc.transpose(2, 0, 1))
            .reshape(KB, 128, T, BS)
            .astype(np.float16)
        )
        h0c = hidden0[c * BS : (c + 1) * BS, :]  # [BS, H]
        h0t = (
            np.ascontiguousarray(h0c.T)
            .reshape(KB, 128, BS)
            .transpose(1, 0, 2)
            .reshape(128, KB * BS)
        )
        in_maps.append(
            {
                "xt16": xt16,
                "wih": wih,
                "whh": whh,
                "biasr": biasr,
                "biasz": biasz,
                "h0t": np.ascontiguousarray(h0t).astype(np.float16),
                "bhn": bhn_t,
                "woutt": wout_t,
                "eye": eye,
            }
        )

    res = run_bass_kernel_spmd(nc, in_maps, list(range(NC)))
    global LAST_RESULT
    LAST_RESULT = res

    ans_f = np.empty((T, B), dtype=np.float32)
    for c in range(NC):
        mb = np.asarray(res.results[c]["mbuf"]).astype(np.float32)  # [128, T, BS]
        ans_f[:, c * BS : (c + 1) * BS] = mb.max(axis=0)
    return np.where(ans_f >= 0.5, 1, -1).astype(np.int32)
